# revision 21
# baseline (speedup 1.0000x reference)
"""Trainium2 Bass kernel for nn_LPModel_85263690760360 (retrieval_knn).

Math: the reference computes, for 6000 queries (left/right of 3000 links),
the 75 smallest hyperboloid sqdists against all 30000 embeddings, and a
margin loss  (sum relu(D_i - topk_vals)) / (2*75*3000).

sqdist is a monotone non-increasing function of the Minkowski product
p = -q0*e0 + q[1:]. e[1:], and is clamped: every candidate with
p >= -(1+EPS) gets exactly sqdist m = arccosh(1+EPS)^2.  Whenever a query
has >= 75 candidates at the clamp, its top-75 values are all exactly m and
its loss contribution collapses to D_i - m (D_i >= GAMMA=1 > m ~ 2.4e-7).

The clamp condition is certified ON DEVICE with a *subset count*: per
query, the number of candidates with p >= -(1+EPS) among a fixed strided
subset of 512 of the 30000 nodes.  A subset count is a lower bound on the
global count, so (subset count >= K_NEG + 64 for every query)  =>  the
collapse is exact.  For the reference inputs the subset counts are ~274
(min 235), i.e. the gate passes with a wide margin; if it ever fails the
kernel falls back to an exact host computation (correct for any input).

Device work per core (8-way shard of the 6000-query axis, 768 q/core):
  - bf16 matmul  Q_shard^T(128x768) x E_sub(128x512) -> P in PSUM,
    6 m-tiles of [128q x 512c], alternating 2 PSUM banks
  - fused threshold+count per m-tile on ACT (sign(p+THR) with free-axis
    accum), giving per-query subset clip counts (the collapse
    certificate).  All counts run on ACT because the DVE tensor_scalar
    accum_out path returns the last element instead of the sum on this
    hardware; DVE scalar_tensor_tensor accum (used for the pair dots)
    works correctly.
  - pair-distance path: D_i for its shard of the 3000 links
    (bf16 dot + f32 arccosh chain with a Newton-refined sqrt)
Host: shard/gather + count-gate check + closed-form assembly; exact numpy
fallback if the gate ever fails (makes kernel() total for any input).

Environment notes (this walrus/axon build):
  - walrus rejects >1 sync-wait per instruction ("Too many sync wait
    commands"): _SplitDrainTileContext splits the Tile kernel-tail drain
    into single-wait drains, and _split_multiwait() post-processes any
    remaining multi-wait instruction the same way.
  - there is no NTFF profile hook, so exec_time_ns is unavailable; the
    runner caches the jitted shard_map callable so repeat calls cost only
    host prep + transfer + dispatch (~0.18 s through the axon tunnel).
"""
import numpy as np
import ml_dtypes
from contextlib import ExitStack

import concourse.bass as bass
import concourse.tile as tile
from concourse import mybir

F32 = mybir.dt.float32
BF16 = mybir.dt.bfloat16

N_NODES = 30000
DIM = 128
T_LINKS = 3000
K_NEG = 75
GAMMA = 1.0
EPS = 1e-7
MAX_SQDIST = 50.0

NCORES = 8
NQ = 2 * T_LINKS                  # 6000 queries
QPC = 768                         # queries per core (6144 total, padded)
MT = QPC // 128                   # 6 query tiles per core
NSUB = 512                        # certificate candidate-subset size
SUB_STRIDE = 58                   # 512*58 = 29696 <= 30000
PAIRS = T_LINKS // NCORES         # 375 pairs per core
PT = 3                            # pair tiles (3*128 = 384 >= 375)

THR = np.float32(1.0 + EPS)                        # theta clip point
M_CONST = float(np.arccosh(np.float64(THR)) ** 2)  # collapsed top-k value
GATE = K_NEG + 64                                  # certificate threshold

LAST_EXEC_NS = None


class _SplitDrainTileContext(tile.TileContext):
    """TileContext whose kernel-tail drain is split into single-wait drains.

    This walrus build caps the number of sync-wait commands one instruction
    may carry; the stock tail drain waits on every active proc at once (one
    wait per engine/DMA-queue semaphore) and is rejected with "Too many sync
    wait commands".  A ladder of SP drains with one wait each executes
    sequentially on SP and is equivalent.
    """

    def _drain_and_barrier(self, tick_clock, wait_clock):
        from concourse.vector_clock import ScopedClock, VectorClock
        from concourse.tile_sem_assignment import N_PROCS

        gc = tick_clock.global_clock
        for p in range(N_PROCS):
            t = gc.peek_next(p) - 1
            if t <= 0:
                continue
            part = VectorClock([t if q == p else 0 for q in range(N_PROCS)])
            d = self.nc.sync.drain()
            wait_clock.add_sem_waits(d.ins, ScopedClock({None: part}))
        self.nc.all_engine_barrier()
        popped = self.nc._tile_sem_poison_stack.pop()
        assert popped is self._sem_poison
        self.nc.clear_and_free_semaphores(list(self.sems.allocated().values()))
        self.nc.all_engine_barrier()


def _split_multiwait(nc):
    """Split multi-wait instructions into single-wait same-engine drains.

    The walrus build in this environment rejects instructions carrying more
    than one sync-wait command ("Too many sync wait commands").  Engine
    queues execute in order, so waiting on A at queue slot n and on B at
    slot n+1 is equivalent to waiting on {A, B} at slot n+1: move all but
    the last wait onto fresh Drain instructions inserted just before the
    offender on the same engine.
    """
    import copy as _copy

    fn = nc.m.functions[0]
    template = None
    for b in fn.blocks:
        for j in b.instructions:
            if type(j).__name__ == "InstDrain":
                template = j
                break
        if template is not None:
            break
    if template is None:
        return 0
    n_split = 0
    for b in fn.blocks:
        insts = b.instructions
        idx = 0
        while idx < len(insts):
            i = insts[idx]
            si = i.sync_info
            if si is not None and si.on_wait and len(si.on_wait) > 1:
                waits = list(si.on_wait)
                for k, w in enumerate(waits[:-1]):
                    nd = _copy.deepcopy(template)
                    nd.name = f"{i.name}-wsplit{k}"
                    nd.engine = i.engine
                    nsi = nd.sync_info
                    nsi.on_wait = [w]
                    nsi.on_update = []
                    nd.sync_info = nsi
                    insts.insert(idx, nd)
                    idx += 1
                si.on_wait = [waits[-1]]
                i.sync_info = si
                n_split += 1
            idx += 1
    return n_split


def _build_nc():
    nc = bass.Bass()

    def reg_const(value):
        t = nc.alloc_sbuf_tensor(f"const-f32-{value}", [128, 1], F32)
        nc.gpsimd.memset(t.ap(), value)
        nc.const_aps.aps[(F32, float(value))] = t.ap()

    reg_const(float(THR))
    reg_const(-1.0)
    nc.all_engine_barrier()

    # query shard + candidate subset fused in one tensor -> one DMA
    qe = nc.dram_tensor("qe", [128, QPC + NSUB], BF16, kind="ExternalInput")
    # pairs, concatenated so one DMA covers both operands (bf16: quantization
    # adds ~1e-3 relative error to the final loss, well inside the 2e-2 gate)
    lr = nc.dram_tensor("lr", [128, 2, PT, 128], BF16, kind="ExternalInput")

    cnt = nc.dram_tensor("cnt", [128, MT], F32, kind="ExternalOutput")
    sqv = nc.dram_tensor("sqv", [128, PT], F32, kind="ExternalOutput")

    with _SplitDrainTileContext(nc) as tc, ExitStack() as ctx:
        weights = ctx.enter_context(tc.tile_pool(name="weights", bufs=1))
        persist = ctx.enter_context(tc.tile_pool(name="persist", bufs=1))
        dpath = ctx.enter_context(tc.tile_pool(name="dpath", bufs=1))
        scratch = ctx.enter_context(tc.tile_pool(name="scratch", bufs=3))
        psA = ctx.enter_context(tc.tile_pool(name="psA", bufs=2, space="PSUM"))
        psD = ctx.enter_context(tc.tile_pool(name="psD", bufs=2, space="PSUM"))

        qe_t = weights.tile([128, QPC + NSUB], BF16)
        lr_t = weights.tile([128, 2, PT, 128], BF16)
        nc.sync.dma_start(out=lr_t, in_=lr[:, :, :, :])
        nc.sync.dma_start(out=qe_t, in_=qe[:, :])
        qT_t = qe_t[:, :QPC]
        eT_t = qe_t[:, QPC:]

        # per-query clip counts, all on ACT via sign-sum (the DVE
        # tensor_scalar accum_out path silently returns the last element
        # instead of the sum on this hardware - measured, not documented)
        a_cnt = persist.tile([128, MT], F32, name="cnt", tag="cnt")

        # ---------------- D path (one core-shard of pairs) ----------------
        d_t = dpath.tile([128, PT], F32)
        for t in range(PT):
            prod = scratch.tile([128, 128], F32, tag="dprod")
            nc.vector.scalar_tensor_tensor(
                out=prod, in0=lr_t[:, 0, t, :], scalar=1.0, in1=lr_t[:, 1, t, :],
                op0=mybir.AluOpType.mult, op1=mybir.AluOpType.mult,
                accum_out=d_t[:, t:t + 1],
            )
        th = dpath.tile([128, PT], F32)
        nc.vector.tensor_scalar(out=th, in0=d_t, scalar1=-1.0, scalar2=float(THR),
                                op0=mybir.AluOpType.mult, op1=mybir.AluOpType.max)
        th2 = dpath.tile([128, PT], F32)
        nc.scalar.activation(out=th2, in_=th, func=mybir.ActivationFunctionType.Square)
        s_t = dpath.tile([128, PT], F32)
        nc.scalar.activation(out=s_t, in_=th2,
                             func=mybir.ActivationFunctionType.Sqrt, bias=-1.0)
        # Newton refine sqrt: s <- 0.5*(s + y/s), y = th2-1
        y_t = dpath.tile([128, PT], F32)
        nc.vector.tensor_scalar(out=y_t, in0=th2, scalar1=-1.0, scalar2=None,
                                op0=mybir.AluOpType.add)
        r_t = dpath.tile([128, PT], F32)
        nc.vector.reciprocal(out=r_t, in_=s_t)
        t1 = dpath.tile([128, PT], F32)
        nc.vector.tensor_mul(out=t1, in0=y_t, in1=r_t)
        s2 = dpath.tile([128, PT], F32)
        nc.vector.tensor_add(out=s2, in0=s_t, in1=t1)
        s3 = dpath.tile([128, PT], F32)
        nc.vector.tensor_scalar(out=s3, in0=s2, scalar1=0.5, scalar2=None,
                                op0=mybir.AluOpType.mult)
        u_t = dpath.tile([128, PT], F32)
        nc.vector.tensor_add(out=u_t, in0=th, in1=s3)
        a_t = dpath.tile([128, PT], F32)
        nc.scalar.activation(out=a_t, in_=u_t, func=mybir.ActivationFunctionType.Ln)
        a2 = dpath.tile([128, PT], F32)
        nc.scalar.activation(out=a2, in_=a_t, func=mybir.ActivationFunctionType.Square)
        sq_t = dpath.tile([128, PT], F32)
        nc.vector.tensor_scalar(out=sq_t, in0=a2, scalar1=float(MAX_SQDIST),
                                scalar2=None, op0=mybir.AluOpType.min)
        nc.sync.dma_start(out=sqv[:, :], in_=sq_t)

        # ---------------- certificate: matmul + threshold-count ------------
        for m in range(MT):
            w = qT_t[:, m * 128:(m + 1) * 128]
            if m % 2 == 0:
                p_ps = psA.tile([128, NSUB], F32, name="pa", tag="pa")
            else:
                p_ps = psD.tile([128, NSUB], F32, name="pd", tag="pd")
            nc.tensor.matmul(p_ps, w, eT_t, start=True, stop=True)
            sg = scratch.tile([128, NSUB], BF16, tag="sg")
            nc.scalar.activation(
                out=sg, in_=p_ps,
                func=mybir.ActivationFunctionType.Sign,
                bias=float(THR), scale=1.0,
                accum_out=a_cnt[:, m:m + 1],
            )

        nc.sync.dma_start(out=cnt[:, :], in_=a_cnt)
    _split_multiwait(nc)
    return nc


_RUNNER = None


def _make_runner():
    """Build nc once and return a cached callable
    (qe_global[1024, QPC+NSUB] bf16, lr_global[1024, 2, PT, 128] bf16)
      -> list of 8 per-core {cnt, sqv} float32 arrays.

    Mirrors concourse.bass_utils.run_bass_kernel_spmd's axon path
    (bass2jax.run_bass_via_pjrt) but hoists the trace/lower/jit out of the
    per-call path so repeat calls skip straight to transfer + execute.
    """
    import jax
    from jax.sharding import Mesh, NamedSharding, PartitionSpec
    from jax.experimental.shard_map import shard_map
    from concourse import bass2jax

    nc = _build_nc()
    bass2jax.install_neuronx_cc_hook()

    partition_name = (nc.partition_id_tensor.name
                      if nc.partition_id_tensor else None)

    in_names, out_names, out_avals, zero_outs = [], [], [], []
    for alloc in nc.m.functions[0].allocations:
        if not isinstance(alloc, mybir.MemoryLocationSet):
            continue
        name = alloc.memorylocations[0].name
        if alloc.kind == "ExternalInput":
            if name != partition_name:
                in_names.append(name)
        elif alloc.kind == "ExternalOutput":
            out_names.append(name)
            shape = tuple(alloc.tensor_shape)
            dtype = mybir.dt.np(alloc.dtype)
            out_avals.append(jax.core.ShapedArray(shape, dtype))
            zero_outs.append(np.zeros((NCORES * shape[0], *shape[1:]), dtype))
    n_params = len(in_names)
    n_outs = len(out_avals)
    all_names = list(in_names) + list(out_names)
    if partition_name is not None:
        all_names.append(partition_name)

    def _body(*args):
        operands = list(args)
        if partition_name is not None:
            operands.append(bass2jax.partition_id_tensor())
        outs = bass2jax._bass_exec_p.bind(
            *operands,
            out_avals=tuple(out_avals),
            in_names=tuple(all_names),
            out_names=tuple(out_names),
            lowering_input_output_aliases=(),
            sim_require_finite=True,
            sim_require_nnan=True,
            nc=nc,
        )
        return tuple(outs)

    devices = jax.devices()[:NCORES]
    assert len(devices) == NCORES
    mesh = Mesh(np.asarray(devices), ("core",))
    spec = PartitionSpec("core")
    in_specs = (spec,) * (n_params + n_outs)
    out_specs = (spec,) * n_outs
    # No donation: both outputs are fully written by the kernel, so the
    # pre-zeroed "output parameter" buffers never need refreshing - keep
    # them resident on device and reuse across calls (saves per-call
    # upload + donation bookkeeping).
    sharded = jax.jit(
        shard_map(_body, mesh=mesh, in_specs=in_specs, out_specs=out_specs,
                  check_rep=False),
        keep_unused=True,
    )
    zdev = [jax.device_put(z, NamedSharding(mesh, spec)) for z in zero_outs]
    jax.block_until_ready(zdev)

    name_to_pos = {n: i for i, n in enumerate(in_names)}

    def run(qe_global, lr_global):
        ins = [None] * n_params
        ins[name_to_pos["qe"]] = qe_global
        ins[name_to_pos["lr"]] = lr_global
        out_arrs = sharded(*ins, *zdev)
        res = []
        for c in range(NCORES):
            res.append({
                name: np.asarray(out_arrs[i]).reshape(
                    NCORES, *out_avals[i].shape)[c]
                for i, name in enumerate(out_names)
            })
        return res

    return run


def _host_fallback(emb, c, links):
    """Exact float64 reference computation on host (safety net)."""
    cs = np.float64(c[0])
    L = emb[links[:, 0]].astype(np.float64)
    R = emb[links[:, 1]].astype(np.float64)
    K = 1.0 / cs

    def sqd(prod):
        theta = np.maximum(-prod / K, 1.0 + EPS)
        return np.minimum(K * np.arccosh(theta) ** 2, MAX_SQDIST)

    d = -L[:, 0] * R[:, 0] + (L[:, 1:] * R[:, 1:]).sum(1)
    D = sqd(d) + GAMMA
    embp = emb.astype(np.float64).copy()
    embp[:, 0] = -embp[:, 0]
    total = 0.0
    for Q in (L, R):
        P = Q @ embp.T
        S = sqd(P)
        S.sort(axis=1)
        topk = S[:, :K_NEG]
        total += np.maximum(D[:, None] - topk, 0.0).sum()
    return np.float32(total / (2.0 * K_NEG * T_LINKS))


def kernel(embeddings, c, train_links):
    global _RUNNER, LAST_EXEC_NS
    emb = np.asarray(embeddings, dtype=np.float32)
    cc = np.asarray(c, dtype=np.float32)
    links = np.asarray(train_links)

    if abs(float(cc[0]) - 1.0) > 1e-12:
        return _host_fallback(emb, cc, links)

    # ---- host-side sharding / layout prep
    L = emb[links[:, 0]]                       # (3000, 128)
    R = emb[links[:, 1]]

    # certificate operands: per-core query shard (bf16) + shared subset
    e_sub = emb[np.arange(NSUB) * SUB_STRIDE].copy()   # (512, 128)
    e_sub[:, 0] = -e_sub[:, 0]                         # fold Minkowski sign
    e_sub_T = np.ascontiguousarray(e_sub.T).astype(ml_dtypes.bfloat16)

    Q = np.concatenate([L, R], axis=0)         # (6000, 128)
    Qp = np.zeros((NCORES * QPC, DIM), np.float32)
    Qp[:NQ] = Q
    # (8, 128, QPC): core c gets queries [c*QPC, (c+1)*QPC) transposed
    QT = Qp.reshape(NCORES, QPC, DIM).transpose(0, 2, 1)
    qe_global = np.empty((NCORES * 128, QPC + NSUB), ml_dtypes.bfloat16)
    qe3 = qe_global.reshape(NCORES, 128, QPC + NSUB)
    qe3[:, :, :QPC] = QT.astype(ml_dtypes.bfloat16)
    qe3[:, :, QPC:] = e_sub_T[None]

    # D-path operands: (8, 128, 2, PT, 128): [c, p, 0, t, k] = Lp[c*375+t*128+p, k]
    Lp = L.copy()
    Lp[:, 0] = -Lp[:, 0]
    lr_pad = np.zeros((2, NCORES, PT, 128, DIM), np.float32)
    lr_pad[0].reshape(-1, DIM)[_pair_scatter_idx()] = Lp
    lr_pad[1].reshape(-1, DIM)[_pair_scatter_idx()] = R
    lr_global = np.ascontiguousarray(
        lr_pad.transpose(1, 3, 0, 2, 4).reshape(NCORES * 128, 2, PT, 128)
    ).astype(ml_dtypes.bfloat16)

    try:
        if _RUNNER is None:
            _RUNNER = _make_runner()
        results = _RUNNER(qe_global, lr_global)
    except Exception:
        return _host_fallback(emb, cc, links)
    LAST_EXEC_NS = None

    # ---- unshard / assemble
    sq_sum = 0.0
    cnt_min = np.inf
    for core in range(NCORES):
        r = results[core]
        # sign-sum to count: cnt = (NSUB + sum_sign)/2
        cnt = (float(NSUB) + r["cnt"].astype(np.float64)) / 2.0  # (128, MT)
        qbase = core * QPC
        nvalid = min(max(NQ - qbase, 0), QPC)
        if nvalid > 0:
            valid = cnt.T.reshape(-1)[:nvalid]
            cnt_min = min(cnt_min, valid.min())
        s = r["sqv"].astype(np.float64).T.reshape(-1)[:PAIRS]
        sq_sum += s.sum()

    if cnt_min < GATE:
        # top-k collapse not certified for some query -> exact fallback
        return _host_fallback(emb, cc, links)

    loss = sq_sum / T_LINKS + GAMMA - M_CONST  # mean(D) - m
    return np.float32(loss)


_PAIR_IDX = None


def _pair_scatter_idx():
    """Flat indices into (NCORES*PT*128) for pair i -> core i//375,
    tile (i%375)//128, partition (i%375)%128."""
    global _PAIR_IDX
    if _PAIR_IDX is None:
        i = np.arange(T_LINKS)
        core, rem = i // PAIRS, i % PAIRS
        _PAIR_IDX = core * (PT * 128) + rem
    return _PAIR_IDX


# revision 22
# speedup vs baseline: 1.0070x; 1.0070x over previous
"""Trainium2 Bass kernel for nn_LPModel_85263690760360 (retrieval_knn).

Math: the reference computes, for 6000 queries (left/right of 3000 links),
the 75 smallest hyperboloid sqdists against all 30000 embeddings, and a
margin loss  (sum relu(D_i - topk_vals)) / (2*75*3000).

sqdist is a monotone non-increasing function of the Minkowski product
p = -q0*e0 + q[1:]. e[1:], and is clamped: every candidate with
p >= -(1+EPS) gets exactly sqdist m = arccosh(1+EPS)^2.  Whenever a query
has >= 75 candidates at the clamp, its top-75 values are all exactly m and
its loss contribution collapses to D_i - m (D_i >= GAMMA=1 > m ~ 2.4e-7).

The clamp condition is certified ON DEVICE with a *subset count*: per
query, the number of candidates with p >= -(1+EPS) among a fixed strided
subset of 512 of the 30000 nodes.  A subset count is a lower bound on the
global count, so (subset count >= K_NEG + 64 for every query)  =>  the
collapse is exact.  For the reference inputs the subset counts are ~274
(min 235), i.e. the gate passes with a wide margin; if it ever fails the
kernel falls back to an exact host computation (correct for any input).

Device work per core (8-way shard of the 6000-query axis, 768 q/core):
  - bf16 matmul  Q_shard^T(128x768) x E_sub(128x512) -> P in PSUM,
    6 m-tiles of [128q x 512c], alternating 2 PSUM banks
  - fused threshold+count per m-tile on ACT (sign(p+THR) with free-axis
    accum), giving per-query subset clip counts (the collapse
    certificate).  All counts run on ACT because the DVE tensor_scalar
    accum_out path returns the last element instead of the sum on this
    hardware; DVE scalar_tensor_tensor accum (used for the pair dots)
    works correctly.
  - pair-distance path: D_i for its shard of the 3000 links
    (bf16 dot + f32 arccosh chain with a Newton-refined sqrt)
Host: shard/gather + count-gate check + closed-form assembly; exact numpy
fallback if the gate ever fails (makes kernel() total for any input).

Environment notes (this walrus/axon build):
  - walrus rejects >1 sync-wait per instruction ("Too many sync wait
    commands"): _SplitDrainTileContext splits the Tile kernel-tail drain
    into single-wait drains, and _split_multiwait() post-processes any
    remaining multi-wait instruction the same way.
  - there is no NTFF profile hook, so exec_time_ns is unavailable; the
    runner caches the jitted shard_map callable so repeat calls cost only
    host prep + transfer + dispatch (~0.18 s through the axon tunnel).
"""
import numpy as np
import ml_dtypes
from contextlib import ExitStack

import concourse.bass as bass
import concourse.tile as tile
from concourse import mybir

F32 = mybir.dt.float32
BF16 = mybir.dt.bfloat16

N_NODES = 30000
DIM = 128
T_LINKS = 3000
K_NEG = 75
GAMMA = 1.0
EPS = 1e-7
MAX_SQDIST = 50.0

NCORES = 8
NQ = 2 * T_LINKS                  # 6000 queries
QPC = 768                         # queries per core (6144 total, padded)
MT = QPC // 128                   # 6 query tiles per core
NSUB = 512                        # certificate candidate-subset size
SUB_STRIDE = 58                   # 512*58 = 29696 <= 30000
PAIRS = T_LINKS // NCORES         # 375 pairs per core
PT = 3                            # pair tiles (3*128 = 384 >= 375)

THR = np.float32(1.0 + EPS)                        # theta clip point
M_CONST = float(np.arccosh(np.float64(THR)) ** 2)  # collapsed top-k value
GATE = K_NEG + 64                                  # certificate threshold

LAST_EXEC_NS = None


class _SplitDrainTileContext(tile.TileContext):
    """TileContext whose kernel-tail drain is split into single-wait drains.

    This walrus build caps the number of sync-wait commands one instruction
    may carry; the stock tail drain waits on every active proc at once (one
    wait per engine/DMA-queue semaphore) and is rejected with "Too many sync
    wait commands".  A ladder of SP drains with one wait each executes
    sequentially on SP and is equivalent.
    """

    def _drain_and_barrier(self, tick_clock, wait_clock):
        from concourse.vector_clock import ScopedClock, VectorClock
        from concourse.tile_sem_assignment import N_PROCS

        gc = tick_clock.global_clock
        for p in range(N_PROCS):
            t = gc.peek_next(p) - 1
            if t <= 0:
                continue
            part = VectorClock([t if q == p else 0 for q in range(N_PROCS)])
            d = self.nc.sync.drain()
            wait_clock.add_sem_waits(d.ins, ScopedClock({None: part}))
        self.nc.all_engine_barrier()
        popped = self.nc._tile_sem_poison_stack.pop()
        assert popped is self._sem_poison
        self.nc.clear_and_free_semaphores(list(self.sems.allocated().values()))
        self.nc.all_engine_barrier()


def _split_multiwait(nc):
    """Split multi-wait instructions into single-wait same-engine drains.

    The walrus build in this environment rejects instructions carrying more
    than one sync-wait command ("Too many sync wait commands").  Engine
    queues execute in order, so waiting on A at queue slot n and on B at
    slot n+1 is equivalent to waiting on {A, B} at slot n+1: move all but
    the last wait onto fresh Drain instructions inserted just before the
    offender on the same engine.
    """
    import copy as _copy

    fn = nc.m.functions[0]
    template = None
    for b in fn.blocks:
        for j in b.instructions:
            if type(j).__name__ == "InstDrain":
                template = j
                break
        if template is not None:
            break
    if template is None:
        return 0
    n_split = 0
    for b in fn.blocks:
        insts = b.instructions
        idx = 0
        while idx < len(insts):
            i = insts[idx]
            si = i.sync_info
            if si is not None and si.on_wait and len(si.on_wait) > 1:
                waits = list(si.on_wait)
                for k, w in enumerate(waits[:-1]):
                    nd = _copy.deepcopy(template)
                    nd.name = f"{i.name}-wsplit{k}"
                    nd.engine = i.engine
                    nsi = nd.sync_info
                    nsi.on_wait = [w]
                    nsi.on_update = []
                    nd.sync_info = nsi
                    insts.insert(idx, nd)
                    idx += 1
                si.on_wait = [waits[-1]]
                i.sync_info = si
                n_split += 1
            idx += 1
    return n_split


def _build_nc():
    nc = bass.Bass()

    def reg_const(value):
        t = nc.alloc_sbuf_tensor(f"const-f32-{value}", [128, 1], F32)
        nc.gpsimd.memset(t.ap(), value)
        nc.const_aps.aps[(F32, float(value))] = t.ap()

    reg_const(float(THR))
    reg_const(-1.0)
    nc.all_engine_barrier()

    # query shard + candidate subset fused in one tensor -> one DMA
    qe = nc.dram_tensor("qe", [128, QPC + NSUB], BF16, kind="ExternalInput")
    # pairs, concatenated so one DMA covers both operands (bf16: quantization
    # adds ~1e-3 relative error to the final loss, well inside the 2e-2 gate)
    lr = nc.dram_tensor("lr", [128, 2, PT, 128], BF16, kind="ExternalInput")

    cnt = nc.dram_tensor("cnt", [128, MT], F32, kind="ExternalOutput")
    sqv = nc.dram_tensor("sqv", [128, PT], F32, kind="ExternalOutput")

    with _SplitDrainTileContext(nc) as tc, ExitStack() as ctx:
        weights = ctx.enter_context(tc.tile_pool(name="weights", bufs=1))
        persist = ctx.enter_context(tc.tile_pool(name="persist", bufs=1))
        dpath = ctx.enter_context(tc.tile_pool(name="dpath", bufs=1))
        scratch = ctx.enter_context(tc.tile_pool(name="scratch", bufs=3))
        psA = ctx.enter_context(tc.tile_pool(name="psA", bufs=2, space="PSUM"))
        psD = ctx.enter_context(tc.tile_pool(name="psD", bufs=2, space="PSUM"))

        qe_t = weights.tile([128, QPC + NSUB], BF16)
        lr_t = weights.tile([128, 2, PT, 128], BF16)
        nc.sync.dma_start(out=lr_t, in_=lr[:, :, :, :])
        nc.sync.dma_start(out=qe_t, in_=qe[:, :])
        qT_t = qe_t[:, :QPC]
        eT_t = qe_t[:, QPC:]

        # per-query clip counts, all on ACT via sign-sum (the DVE
        # tensor_scalar accum_out path silently returns the last element
        # instead of the sum on this hardware - measured, not documented)
        a_cnt = persist.tile([128, MT], F32, name="cnt", tag="cnt")

        # ---------------- D path (one core-shard of pairs) ----------------
        d_t = dpath.tile([128, PT], F32)
        for t in range(PT):
            prod = scratch.tile([128, 128], F32, tag="dprod")
            nc.vector.scalar_tensor_tensor(
                out=prod, in0=lr_t[:, 0, t, :], scalar=1.0, in1=lr_t[:, 1, t, :],
                op0=mybir.AluOpType.mult, op1=mybir.AluOpType.mult,
                accum_out=d_t[:, t:t + 1],
            )
        th = dpath.tile([128, PT], F32)
        nc.vector.tensor_scalar(out=th, in0=d_t, scalar1=-1.0, scalar2=float(THR),
                                op0=mybir.AluOpType.mult, op1=mybir.AluOpType.max)
        th2 = dpath.tile([128, PT], F32)
        nc.scalar.activation(out=th2, in_=th, func=mybir.ActivationFunctionType.Square)
        s_t = dpath.tile([128, PT], F32)
        nc.scalar.activation(out=s_t, in_=th2,
                             func=mybir.ActivationFunctionType.Sqrt, bias=-1.0)
        # Newton refine sqrt: s <- 0.5*(s + y/s), y = th2-1
        y_t = dpath.tile([128, PT], F32)
        nc.vector.tensor_scalar(out=y_t, in0=th2, scalar1=-1.0, scalar2=None,
                                op0=mybir.AluOpType.add)
        r_t = dpath.tile([128, PT], F32)
        nc.vector.reciprocal(out=r_t, in_=s_t)
        t1 = dpath.tile([128, PT], F32)
        nc.vector.tensor_mul(out=t1, in0=y_t, in1=r_t)
        s2 = dpath.tile([128, PT], F32)
        nc.vector.tensor_add(out=s2, in0=s_t, in1=t1)
        s3 = dpath.tile([128, PT], F32)
        nc.vector.tensor_scalar(out=s3, in0=s2, scalar1=0.5, scalar2=None,
                                op0=mybir.AluOpType.mult)
        u_t = dpath.tile([128, PT], F32)
        nc.vector.tensor_add(out=u_t, in0=th, in1=s3)
        a_t = dpath.tile([128, PT], F32)
        nc.scalar.activation(out=a_t, in_=u_t, func=mybir.ActivationFunctionType.Ln)
        a2 = dpath.tile([128, PT], F32)
        nc.scalar.activation(out=a2, in_=a_t, func=mybir.ActivationFunctionType.Square)
        sq_t = dpath.tile([128, PT], F32)
        nc.vector.tensor_scalar(out=sq_t, in0=a2, scalar1=float(MAX_SQDIST),
                                scalar2=None, op0=mybir.AluOpType.min)
        nc.sync.dma_start(out=sqv[:, :], in_=sq_t)

        # ---------------- certificate: matmul + threshold-count ------------
        for m in range(MT):
            w = qT_t[:, m * 128:(m + 1) * 128]
            if m % 2 == 0:
                p_ps = psA.tile([128, NSUB], F32, name="pa", tag="pa")
            else:
                p_ps = psD.tile([128, NSUB], F32, name="pd", tag="pd")
            nc.tensor.matmul(p_ps, w, eT_t, start=True, stop=True)
            sg = scratch.tile([128, NSUB], BF16, tag="sg")
            nc.scalar.activation(
                out=sg, in_=p_ps,
                func=mybir.ActivationFunctionType.Sign,
                bias=float(THR), scale=1.0,
                accum_out=a_cnt[:, m:m + 1],
            )

        nc.sync.dma_start(out=cnt[:, :], in_=a_cnt)
    _split_multiwait(nc)
    return nc


_RUNNER = None


def _make_runner():
    """Build nc once and return a cached callable
    (qe_global[1024, QPC+NSUB] bf16, lr_global[1024, 2, PT, 128] bf16)
      -> list of 8 per-core {cnt, sqv} float32 arrays.

    Mirrors concourse.bass_utils.run_bass_kernel_spmd's axon path
    (bass2jax.run_bass_via_pjrt) but hoists the trace/lower/jit out of the
    per-call path so repeat calls skip straight to transfer + execute.
    """
    import jax
    from jax.sharding import Mesh, NamedSharding, PartitionSpec
    from jax.experimental.shard_map import shard_map
    from concourse import bass2jax

    nc = _build_nc()
    bass2jax.install_neuronx_cc_hook()

    partition_name = (nc.partition_id_tensor.name
                      if nc.partition_id_tensor else None)

    in_names, out_names, out_avals, zero_outs = [], [], [], []
    for alloc in nc.m.functions[0].allocations:
        if not isinstance(alloc, mybir.MemoryLocationSet):
            continue
        name = alloc.memorylocations[0].name
        if alloc.kind == "ExternalInput":
            if name != partition_name:
                in_names.append(name)
        elif alloc.kind == "ExternalOutput":
            out_names.append(name)
            shape = tuple(alloc.tensor_shape)
            dtype = mybir.dt.np(alloc.dtype)
            out_avals.append(jax.core.ShapedArray(shape, dtype))
            zero_outs.append(np.zeros((NCORES * shape[0], *shape[1:]), dtype))
    n_params = len(in_names)
    n_outs = len(out_avals)
    all_names = list(in_names) + list(out_names)
    if partition_name is not None:
        all_names.append(partition_name)

    def _body(*args):
        operands = list(args)
        if partition_name is not None:
            operands.append(bass2jax.partition_id_tensor())
        outs = bass2jax._bass_exec_p.bind(
            *operands,
            out_avals=tuple(out_avals),
            in_names=tuple(all_names),
            out_names=tuple(out_names),
            lowering_input_output_aliases=(),
            sim_require_finite=True,
            sim_require_nnan=True,
            nc=nc,
        )
        return tuple(outs)

    devices = jax.devices()[:NCORES]
    assert len(devices) == NCORES
    mesh = Mesh(np.asarray(devices), ("core",))
    spec = PartitionSpec("core")
    in_specs = (spec,) * (n_params + n_outs)
    out_specs = (spec,) * n_outs
    # No donation: both outputs are fully written by the kernel, so the
    # pre-zeroed "output parameter" buffers never need refreshing - keep
    # them resident on device and reuse across calls (saves per-call
    # upload + donation bookkeeping).
    sharded = jax.jit(
        shard_map(_body, mesh=mesh, in_specs=in_specs, out_specs=out_specs,
                  check_rep=False),
        keep_unused=True,
    )
    zdev = [jax.device_put(z, NamedSharding(mesh, spec)) for z in zero_outs]
    jax.block_until_ready(zdev)

    name_to_pos = {n: i for i, n in enumerate(in_names)}

    def run(qe_global, lr_global):
        ins = [None] * n_params
        ins[name_to_pos["qe"]] = qe_global
        ins[name_to_pos["lr"]] = lr_global
        out_arrs = sharded(*ins, *zdev)
        res = []
        for c in range(NCORES):
            res.append({
                name: np.asarray(out_arrs[i]).reshape(
                    NCORES, *out_avals[i].shape)[c]
                for i, name in enumerate(out_names)
            })
        return res

    return run


def _host_fallback(emb, c, links):
    """Exact reference computation on host (safety net).

    sqdist is monotone non-increasing in the Minkowski product p, so the 75
    smallest sqdists are the 75 largest p: select them with an O(N) f32
    partition, then evaluate the arccosh chain in f64 on just those.
    Bit-identical to the full f64 sort on the reference inputs, ~24x faster
    (~2 s vs ~50 s).
    """
    cs = np.float64(c[0])
    L = emb[links[:, 0]].astype(np.float64)
    R = emb[links[:, 1]].astype(np.float64)
    K = 1.0 / cs

    def sqd(prod):
        theta = np.maximum(-prod / K, 1.0 + EPS)
        return np.minimum(K * np.arccosh(theta) ** 2, MAX_SQDIST)

    d = -L[:, 0] * R[:, 0] + (L[:, 1:] * R[:, 1:]).sum(1)
    D = sqd(d) + GAMMA
    embp32 = emb.copy()
    embp32[:, 0] = -embp32[:, 0]
    total = 0.0
    for Q32 in (emb[links[:, 0]], emb[links[:, 1]]):
        P32 = Q32 @ embp32.T                                   # (3000, 30000)
        topp = -np.partition(-P32, K_NEG - 1, axis=1)[:, :K_NEG]
        S = sqd(topp.astype(np.float64))
        total += np.maximum(D[:, None] - S, 0.0).sum()
    return np.float32(total / (2.0 * K_NEG * T_LINKS))


def kernel(embeddings, c, train_links):
    global _RUNNER, LAST_EXEC_NS
    emb = np.asarray(embeddings, dtype=np.float32)
    cc = np.asarray(c, dtype=np.float32)
    links = np.asarray(train_links)

    if abs(float(cc[0]) - 1.0) > 1e-12:
        return _host_fallback(emb, cc, links)

    # ---- host-side sharding / layout prep
    L = emb[links[:, 0]]                       # (3000, 128)
    R = emb[links[:, 1]]

    # certificate operands: per-core query shard (bf16) + shared subset
    e_sub = emb[np.arange(NSUB) * SUB_STRIDE].copy()   # (512, 128)
    e_sub[:, 0] = -e_sub[:, 0]                         # fold Minkowski sign
    e_sub_T = np.ascontiguousarray(e_sub.T).astype(ml_dtypes.bfloat16)

    Q = np.concatenate([L, R], axis=0)         # (6000, 128)
    Qp = np.zeros((NCORES * QPC, DIM), np.float32)
    Qp[:NQ] = Q
    # (8, 128, QPC): core c gets queries [c*QPC, (c+1)*QPC) transposed
    QT = Qp.reshape(NCORES, QPC, DIM).transpose(0, 2, 1)
    qe_global = np.empty((NCORES * 128, QPC + NSUB), ml_dtypes.bfloat16)
    qe3 = qe_global.reshape(NCORES, 128, QPC + NSUB)
    qe3[:, :, :QPC] = QT.astype(ml_dtypes.bfloat16)
    qe3[:, :, QPC:] = e_sub_T[None]

    # D-path operands: (8, 128, 2, PT, 128): [c, p, 0, t, k] = Lp[c*375+t*128+p, k]
    Lp = L.copy()
    Lp[:, 0] = -Lp[:, 0]
    lr_pad = np.zeros((2, NCORES, PT, 128, DIM), np.float32)
    lr_pad[0].reshape(-1, DIM)[_pair_scatter_idx()] = Lp
    lr_pad[1].reshape(-1, DIM)[_pair_scatter_idx()] = R
    lr_global = np.ascontiguousarray(
        lr_pad.transpose(1, 3, 0, 2, 4).reshape(NCORES * 128, 2, PT, 128)
    ).astype(ml_dtypes.bfloat16)

    try:
        if _RUNNER is None:
            _RUNNER = _make_runner()
        results = _RUNNER(qe_global, lr_global)
    except Exception:
        return _host_fallback(emb, cc, links)
    LAST_EXEC_NS = None

    # ---- unshard / assemble
    sq_sum = 0.0
    cnt_min = np.inf
    for core in range(NCORES):
        r = results[core]
        # sign-sum to count: cnt = (NSUB + sum_sign)/2
        cnt = (float(NSUB) + r["cnt"].astype(np.float64)) / 2.0  # (128, MT)
        qbase = core * QPC
        nvalid = min(max(NQ - qbase, 0), QPC)
        if nvalid > 0:
            valid = cnt.T.reshape(-1)[:nvalid]
            cnt_min = min(cnt_min, valid.min())
        s = r["sqv"].astype(np.float64).T.reshape(-1)[:PAIRS]
        sq_sum += s.sum()

    if cnt_min < GATE:
        # top-k collapse not certified for some query -> exact fallback
        return _host_fallback(emb, cc, links)

    loss = sq_sum / T_LINKS + GAMMA - M_CONST  # mean(D) - m
    return np.float32(loss)


_PAIR_IDX = None


def _pair_scatter_idx():
    """Flat indices into (NCORES*PT*128) for pair i -> core i//375,
    tile (i%375)//128, partition (i%375)%128."""
    global _PAIR_IDX
    if _PAIR_IDX is None:
        i = np.arange(T_LINKS)
        core, rem = i // PAIRS, i % PAIRS
        _PAIR_IDX = core * (PT * 128) + rem
    return _PAIR_IDX


# revision 31
# speedup vs baseline: 1.7502x; 1.7380x over previous
"""Trainium2 Bass kernel for nn_LPModel_85263690760360 (retrieval_knn).

Math: the reference computes, for 6000 queries (left/right of 3000 links),
the 75 smallest hyperboloid sqdists against all 30000 embeddings, and a
margin loss  (sum relu(D_i - topk_vals)) / (2*75*3000).

sqdist is a monotone non-increasing function of the Minkowski product
p = -q0*e0 + q[1:]. e[1:], and is clamped: every candidate with
p >= -(1+EPS) gets exactly sqdist m = arccosh(1+EPS)^2.  Whenever a query
has >= 75 candidates at the clamp, its top-75 values are all exactly m and
its loss contribution collapses to D_i - m (D_i >= GAMMA=1 > m ~ 2.4e-7).

The clamp condition is certified ON DEVICE with a *subset count*: per
query, the number of candidates with p >= -(1+EPS) among a fixed strided
subset of 512 of the 30000 nodes.  A subset count is a lower bound on the
global count, so (subset count >= K_NEG + 64 for every query)  =>  the
collapse is exact.  For the reference inputs the subset counts are ~274
(min 235), i.e. the gate passes with a wide margin; if it ever fails the
kernel falls back to an exact host computation (correct for any input).

Device work per core (8-way shard of the 6000-query axis, 768 q/core):
  - bf16 matmul  Q_shard^T(128x768) x E_sub(128x512) -> P in PSUM,
    6 m-tiles of [128q x 512c], alternating 2 PSUM banks
  - fused threshold+count per m-tile on ACT (sign(p+THR) with free-axis
    accum), giving per-query subset clip counts (the collapse
    certificate).  All counts run on ACT because the DVE tensor_scalar
    accum_out path returns the last element instead of the sum on this
    hardware; DVE scalar_tensor_tensor accum (used for the pair dots)
    works correctly.
  - pair-distance path: D_i for its shard of the 3000 links
    (bf16 dot + f32 arccosh chain with a Newton-refined sqrt)
Host: shard/gather + count-gate check + closed-form assembly; exact numpy
fallback if the gate ever fails (makes kernel() total for any input).

Environment notes (this walrus/axon build):
  - walrus rejects >1 sync-wait per instruction ("Too many sync wait
    commands"): _SplitDrainTileContext splits the Tile kernel-tail drain
    into single-wait drains, and _split_multiwait() post-processes any
    remaining multi-wait instruction the same way.
  - there is no NTFF profile hook, so exec_time_ns is unavailable; the
    runner caches the jitted shard_map callable so repeat calls cost only
    host prep + transfer + dispatch (~0.18 s through the axon tunnel).
"""
import numpy as np
import ml_dtypes
from contextlib import ExitStack

import concourse.bass as bass
import concourse.tile as tile
from concourse import mybir

F32 = mybir.dt.float32
BF16 = mybir.dt.bfloat16

N_NODES = 30000
DIM = 128
T_LINKS = 3000
K_NEG = 75
GAMMA = 1.0
EPS = 1e-7
MAX_SQDIST = 50.0

NCORES = 8
NQ = 2 * T_LINKS                  # 6000 queries
QPC = 768                         # queries per core (6144 total, padded)
MT = QPC // 128                   # 6 query tiles per core
NSUB = 512                        # certificate candidate-subset size
SUB_STRIDE = 58                   # 512*58 = 29696 <= 30000
PAIRS = T_LINKS // NCORES         # 375 pairs per core
PT = 3                            # pair tiles (3*128 = 384 >= 375)

THR = np.float32(1.0 + EPS)                        # theta clip point
M_CONST = float(np.arccosh(np.float64(THR)) ** 2)  # collapsed top-k value
GATE = K_NEG + 64                                  # certificate threshold
QELR_W = QPC + NSUB + 2 * PT * 128                 # fused input width (2048)

LAST_EXEC_NS = None


class _SplitDrainTileContext(tile.TileContext):
    """TileContext whose kernel-tail drain is split into single-wait drains.

    This walrus build caps the number of sync-wait commands one instruction
    may carry; the stock tail drain waits on every active proc at once (one
    wait per engine/DMA-queue semaphore) and is rejected with "Too many sync
    wait commands".  A ladder of SP drains with one wait each executes
    sequentially on SP and is equivalent.
    """

    def _drain_and_barrier(self, tick_clock, wait_clock):
        from concourse.vector_clock import ScopedClock, VectorClock
        from concourse.tile_sem_assignment import N_PROCS

        gc = tick_clock.global_clock
        for p in range(N_PROCS):
            t = gc.peek_next(p) - 1
            if t <= 0:
                continue
            part = VectorClock([t if q == p else 0 for q in range(N_PROCS)])
            d = self.nc.sync.drain()
            wait_clock.add_sem_waits(d.ins, ScopedClock({None: part}))
        self.nc.all_engine_barrier()
        popped = self.nc._tile_sem_poison_stack.pop()
        assert popped is self._sem_poison
        self.nc.clear_and_free_semaphores(list(self.sems.allocated().values()))
        self.nc.all_engine_barrier()


def _split_multiwait(nc):
    """Split multi-wait instructions into single-wait same-engine drains.

    The walrus build in this environment rejects instructions carrying more
    than one sync-wait command ("Too many sync wait commands").  Engine
    queues execute in order, so waiting on A at queue slot n and on B at
    slot n+1 is equivalent to waiting on {A, B} at slot n+1: move all but
    the last wait onto fresh Drain instructions inserted just before the
    offender on the same engine.
    """
    import copy as _copy

    fn = nc.m.functions[0]
    template = None
    for b in fn.blocks:
        for j in b.instructions:
            if type(j).__name__ == "InstDrain":
                template = j
                break
        if template is not None:
            break
    if template is None:
        return 0
    n_split = 0
    for b in fn.blocks:
        insts = b.instructions
        idx = 0
        while idx < len(insts):
            i = insts[idx]
            si = i.sync_info
            if si is not None and si.on_wait and len(si.on_wait) > 1:
                waits = list(si.on_wait)
                for k, w in enumerate(waits[:-1]):
                    nd = _copy.deepcopy(template)
                    nd.name = f"{i.name}-wsplit{k}"
                    nd.engine = i.engine
                    nsi = nd.sync_info
                    nsi.on_wait = [w]
                    nsi.on_update = []
                    nd.sync_info = nsi
                    insts.insert(idx, nd)
                    idx += 1
                si.on_wait = [waits[-1]]
                i.sync_info = si
                n_split += 1
            idx += 1
    return n_split


def _build_nc():
    nc = bass.Bass()

    def reg_const(value):
        t = nc.alloc_sbuf_tensor(f"const-f32-{value}", [128, 1], F32)
        nc.gpsimd.memset(t.ap(), value)
        nc.const_aps.aps[(F32, float(value))] = t.ap()

    reg_const(float(THR))
    reg_const(-1.0)
    nc.all_engine_barrier()

    # query shard + candidate subset + pair operands fused in ONE bf16
    # tensor: a single host array / transfer / DMA.  Columns:
    #   [0, QPC)                 Q_shard^T
    #   [QPC, QPC+NSUB)          E_sub^T (Minkowski sign folded)
    #   [QPC+NSUB, +2*PT*128)    pair tiles, col = base + which*PT*128 + t*128 + k
    # (bf16 pair operands add ~2e-5 relative error to the final loss)
    LRBASE = QPC + NSUB
    qelr = nc.dram_tensor("qelr", [128, LRBASE + 2 * PT * 128], BF16,
                          kind="ExternalInput")

    # counts (cols 0..MT-1, ACT) and pair sqdists (cols MT..MT+PT-1, DVE)
    # fused in one output -> one result array through PJRT
    res = nc.dram_tensor("res", [128, MT + PT], F32, kind="ExternalOutput")

    with _SplitDrainTileContext(nc) as tc, ExitStack() as ctx:
        weights = ctx.enter_context(tc.tile_pool(name="weights", bufs=1))
        persist = ctx.enter_context(tc.tile_pool(name="persist", bufs=1))
        dpath = ctx.enter_context(tc.tile_pool(name="dpath", bufs=1))
        scratch = ctx.enter_context(tc.tile_pool(name="scratch", bufs=3))
        psA = ctx.enter_context(tc.tile_pool(name="psA", bufs=2, space="PSUM"))
        psD = ctx.enter_context(tc.tile_pool(name="psD", bufs=2, space="PSUM"))

        qelr_t = weights.tile([128, LRBASE + 2 * PT * 128], BF16)
        nc.sync.dma_start(out=qelr_t, in_=qelr[:, :])
        qT_t = qelr_t[:, :QPC]
        eT_t = qelr_t[:, QPC:QPC + NSUB]

        def lr_tile(which, t):
            c0 = LRBASE + which * PT * 128 + t * 128
            return qelr_t[:, c0:c0 + 128]

        # fused output: per-query clip counts (ACT via sign-sum; the DVE
        # tensor_scalar accum_out path silently returns the last element
        # instead of the sum on this hardware) + pair sqdists (DVE)
        a_out = persist.tile([128, MT + PT], F32, name="res", tag="res")
        a_cnt = a_out[:, :MT]

        # ---------------- D path (one core-shard of pairs) ----------------
        d_t = dpath.tile([128, PT], F32)
        for t in range(PT):
            prod = scratch.tile([128, 128], F32, tag="dprod")
            nc.vector.scalar_tensor_tensor(
                out=prod, in0=lr_tile(0, t), scalar=1.0, in1=lr_tile(1, t),
                op0=mybir.AluOpType.mult, op1=mybir.AluOpType.mult,
                accum_out=d_t[:, t:t + 1],
            )
        th = dpath.tile([128, PT], F32)
        nc.vector.tensor_scalar(out=th, in0=d_t, scalar1=-1.0, scalar2=float(THR),
                                op0=mybir.AluOpType.mult, op1=mybir.AluOpType.max)
        th2 = dpath.tile([128, PT], F32)
        nc.scalar.activation(out=th2, in_=th, func=mybir.ActivationFunctionType.Square)
        s_t = dpath.tile([128, PT], F32)
        nc.scalar.activation(out=s_t, in_=th2,
                             func=mybir.ActivationFunctionType.Sqrt, bias=-1.0)
        # Newton refine sqrt: s <- 0.5*(s + y/s), y = th2-1
        y_t = dpath.tile([128, PT], F32)
        nc.vector.tensor_scalar(out=y_t, in0=th2, scalar1=-1.0, scalar2=None,
                                op0=mybir.AluOpType.add)
        r_t = dpath.tile([128, PT], F32)
        nc.vector.reciprocal(out=r_t, in_=s_t)
        t1 = dpath.tile([128, PT], F32)
        nc.vector.tensor_mul(out=t1, in0=y_t, in1=r_t)
        s2 = dpath.tile([128, PT], F32)
        nc.vector.tensor_add(out=s2, in0=s_t, in1=t1)
        s3 = dpath.tile([128, PT], F32)
        nc.vector.tensor_scalar(out=s3, in0=s2, scalar1=0.5, scalar2=None,
                                op0=mybir.AluOpType.mult)
        u_t = dpath.tile([128, PT], F32)
        nc.vector.tensor_add(out=u_t, in0=th, in1=s3)
        a_t = dpath.tile([128, PT], F32)
        nc.scalar.activation(out=a_t, in_=u_t, func=mybir.ActivationFunctionType.Ln)
        a2 = dpath.tile([128, PT], F32)
        nc.scalar.activation(out=a2, in_=a_t, func=mybir.ActivationFunctionType.Square)
        nc.vector.tensor_scalar(out=a_out[:, MT:], in0=a2,
                                scalar1=float(MAX_SQDIST),
                                scalar2=None, op0=mybir.AluOpType.min)

        # ---------------- certificate: matmul + threshold-count ------------
        for m in range(MT):
            w = qT_t[:, m * 128:(m + 1) * 128]
            if m % 2 == 0:
                p_ps = psA.tile([128, NSUB], F32, name="pa", tag="pa")
            else:
                p_ps = psD.tile([128, NSUB], F32, name="pd", tag="pd")
            nc.tensor.matmul(p_ps, w, eT_t, start=True, stop=True)
            sg = scratch.tile([128, NSUB], BF16, tag="sg")
            nc.scalar.activation(
                out=sg, in_=p_ps,
                func=mybir.ActivationFunctionType.Sign,
                bias=float(THR), scale=1.0,
                accum_out=a_cnt[:, m:m + 1],
            )

        nc.sync.dma_start(out=res[:, :], in_=a_out)
    _split_multiwait(nc)
    return nc


_RUNNER = None


def _make_runner():
    """Build nc once and return a cached callable
    (qelr_global[1024, QPC+NSUB+2*PT*128] bf16)
      -> list of 8 per-core {res} float32 arrays.

    Mirrors concourse.bass_utils.run_bass_kernel_spmd's axon path
    (bass2jax.run_bass_via_pjrt) but hoists the trace/lower/jit out of the
    per-call path so repeat calls skip straight to transfer + execute.
    """
    import jax
    from jax.sharding import Mesh, NamedSharding, PartitionSpec
    from jax.experimental.shard_map import shard_map
    from concourse import bass2jax

    nc = _build_nc()
    bass2jax.install_neuronx_cc_hook()

    partition_name = (nc.partition_id_tensor.name
                      if nc.partition_id_tensor else None)

    in_names, out_names, out_avals, zero_outs = [], [], [], []
    for alloc in nc.m.functions[0].allocations:
        if not isinstance(alloc, mybir.MemoryLocationSet):
            continue
        name = alloc.memorylocations[0].name
        if alloc.kind == "ExternalInput":
            if name != partition_name:
                in_names.append(name)
        elif alloc.kind == "ExternalOutput":
            out_names.append(name)
            shape = tuple(alloc.tensor_shape)
            dtype = mybir.dt.np(alloc.dtype)
            out_avals.append(jax.core.ShapedArray(shape, dtype))
            zero_outs.append(np.zeros((NCORES * shape[0], *shape[1:]), dtype))
    n_params = len(in_names)
    n_outs = len(out_avals)
    all_names = list(in_names) + list(out_names)
    if partition_name is not None:
        all_names.append(partition_name)

    def _body(*args):
        operands = list(args)
        if partition_name is not None:
            operands.append(bass2jax.partition_id_tensor())
        outs = bass2jax._bass_exec_p.bind(
            *operands,
            out_avals=tuple(out_avals),
            in_names=tuple(all_names),
            out_names=tuple(out_names),
            lowering_input_output_aliases=(),
            sim_require_finite=True,
            sim_require_nnan=True,
            nc=nc,
        )
        return tuple(outs)

    devices = jax.devices()[:NCORES]
    assert len(devices) == NCORES
    mesh = Mesh(np.asarray(devices), ("core",))
    spec = PartitionSpec("core")
    in_specs = (spec,) * (n_params + n_outs)
    out_specs = (spec,) * n_outs
    # No donation: both outputs are fully written by the kernel, so the
    # pre-zeroed "output parameter" buffers never need refreshing - keep
    # them resident on device and reuse across calls (saves per-call
    # upload + donation bookkeeping).
    sharded = jax.jit(
        shard_map(_body, mesh=mesh, in_specs=in_specs, out_specs=out_specs,
                  check_rep=False),
        keep_unused=True,
    )
    zdev = [jax.device_put(z, NamedSharding(mesh, spec)) for z in zero_outs]
    jax.block_until_ready(zdev)

    name_to_pos = {n: i for i, n in enumerate(in_names)}

    def run(qelr_global):
        ins = [None] * n_params
        ins[name_to_pos["qelr"]] = qelr_global
        out_arrs = sharded(*ins, *zdev)
        res = []
        for c in range(NCORES):
            res.append({
                name: np.asarray(out_arrs[i]).reshape(
                    NCORES, *out_avals[i].shape)[c]
                for i, name in enumerate(out_names)
            })
        return res

    return run


def _host_fallback(emb, c, links):
    """Exact reference computation on host (safety net).

    sqdist is monotone non-increasing in the Minkowski product p, so the 75
    smallest sqdists are the 75 largest p: select them with an O(N) f32
    partition, then evaluate the arccosh chain in f64 on just those.
    Bit-identical to the full f64 sort on the reference inputs, ~24x faster
    (~2 s vs ~50 s).
    """
    cs = np.float64(c[0])
    L = emb[links[:, 0]].astype(np.float64)
    R = emb[links[:, 1]].astype(np.float64)
    K = 1.0 / cs

    def sqd(prod):
        theta = np.maximum(-prod / K, 1.0 + EPS)
        return np.minimum(K * np.arccosh(theta) ** 2, MAX_SQDIST)

    d = -L[:, 0] * R[:, 0] + (L[:, 1:] * R[:, 1:]).sum(1)
    D = sqd(d) + GAMMA
    embp32 = emb.copy()
    embp32[:, 0] = -embp32[:, 0]
    total = 0.0
    for Q32 in (emb[links[:, 0]], emb[links[:, 1]]):
        P32 = Q32 @ embp32.T                                   # (3000, 30000)
        topp = -np.partition(-P32, K_NEG - 1, axis=1)[:, :K_NEG]
        S = sqd(topp.astype(np.float64))
        total += np.maximum(D[:, None] - S, 0.0).sum()
    return np.float32(total / (2.0 * K_NEG * T_LINKS))


def kernel(embeddings, c, train_links):
    global _RUNNER, LAST_EXEC_NS
    emb = np.asarray(embeddings, dtype=np.float32)
    cc = np.asarray(c, dtype=np.float32)
    links = np.asarray(train_links)

    if abs(float(cc[0]) - 1.0) > 1e-12:
        return _host_fallback(emb, cc, links)

    # ---- host-side sharding / layout prep
    L = emb[links[:, 0]]                       # (3000, 128)
    R = emb[links[:, 1]]

    # certificate operands: per-core query shard (bf16) + shared subset
    e_sub = emb[np.arange(NSUB) * SUB_STRIDE].copy()   # (512, 128)
    e_sub[:, 0] = -e_sub[:, 0]                         # fold Minkowski sign
    e_sub_T = np.ascontiguousarray(e_sub.T).astype(ml_dtypes.bfloat16)

    Q = np.concatenate([L, R], axis=0)         # (6000, 128)
    Qp = np.zeros((NCORES * QPC, DIM), np.float32)
    Qp[:NQ] = Q
    # (8, 128, QPC): core c gets queries [c*QPC, (c+1)*QPC) transposed
    QT = Qp.reshape(NCORES, QPC, DIM).transpose(0, 2, 1)

    # D-path operands: [c, p, which, t, k] = (Lp|R)[c*375+t*128+p, k]
    Lp = L.copy()
    Lp[:, 0] = -Lp[:, 0]
    lr_pad = np.zeros((2, NCORES, PT, 128, DIM), np.float32)
    lr_pad[0].reshape(-1, DIM)[_pair_scatter_idx()] = Lp
    lr_pad[1].reshape(-1, DIM)[_pair_scatter_idx()] = R

    # single fused input array (implicit f32 -> bf16 casts on assignment)
    qelr_global = np.empty((NCORES * 128, QELR_W), ml_dtypes.bfloat16)
    q3 = qelr_global.reshape(NCORES, 128, QELR_W)
    q3[:, :, :QPC] = QT
    q3[:, :, QPC:QPC + NSUB] = e_sub_T[None]
    q3[:, :, QPC + NSUB:] = lr_pad.transpose(1, 3, 0, 2, 4).reshape(
        NCORES, 128, 2 * PT * 128)

    try:
        if _RUNNER is None:
            _RUNNER = _make_runner()
        results = _RUNNER(qelr_global)
    except Exception:
        return _host_fallback(emb, cc, links)
    LAST_EXEC_NS = None

    # ---- unshard / assemble
    sq_sum = 0.0
    cnt_min = np.inf
    for core in range(NCORES):
        r = results[core]["res"]
        # sign-sum to count: cnt = (NSUB + sum_sign)/2
        cnt = (float(NSUB) + r[:, :MT].astype(np.float64)) / 2.0  # (128, MT)
        qbase = core * QPC
        nvalid = min(max(NQ - qbase, 0), QPC)
        if nvalid > 0:
            valid = cnt.T.reshape(-1)[:nvalid]
            cnt_min = min(cnt_min, valid.min())
        s = r[:, MT:].astype(np.float64).T.reshape(-1)[:PAIRS]
        sq_sum += s.sum()

    if cnt_min < GATE:
        # top-k collapse not certified for some query -> exact fallback
        return _host_fallback(emb, cc, links)

    loss = sq_sum / T_LINKS + GAMMA - M_CONST  # mean(D) - m
    return np.float32(loss)


_PAIR_IDX = None


def _pair_scatter_idx():
    """Flat indices into (NCORES*PT*128) for pair i -> core i//375,
    tile (i%375)//128, partition (i%375)%128."""
    global _PAIR_IDX
    if _PAIR_IDX is None:
        i = np.arange(T_LINKS)
        core, rem = i // PAIRS, i % PAIRS
        _PAIR_IDX = core * (PT * 128) + rem
    return _PAIR_IDX


# revision 32
# speedup vs baseline: 1.9347x; 1.1054x over previous
"""Trainium2 Bass kernel for nn_LPModel_85263690760360 (retrieval_knn).

Math: the reference computes, for 6000 queries (left/right of 3000 links),
the 75 smallest hyperboloid sqdists against all 30000 embeddings, and a
margin loss  (sum relu(D_i - topk_vals)) / (2*75*3000).

sqdist is a monotone non-increasing function of the Minkowski product
p = -q0*e0 + q[1:]. e[1:], and is clamped: every candidate with
p >= -(1+EPS) gets exactly sqdist m = arccosh(1+EPS)^2.  Whenever a query
has >= 75 candidates at the clamp, its top-75 values are all exactly m and
its loss contribution collapses to D_i - m (D_i >= GAMMA=1 > m ~ 2.4e-7).

The clamp condition is certified ON DEVICE with a *subset count*: per
query, the number of candidates with p >= -(1+EPS) among a fixed strided
subset of 512 of the 30000 nodes.  A subset count is a lower bound on the
global count, so (subset count >= K_NEG + 64 for every query)  =>  the
collapse is exact.  For the reference inputs the subset counts are ~274
(min 235), i.e. the gate passes with a wide margin; if it ever fails the
kernel falls back to an exact host computation (correct for any input).

Device work per core (8-way shard of the 6000-query axis, 768 q/core):
  - bf16 matmul  Q_shard^T(128x768) x E_sub(128x512) -> P in PSUM,
    6 m-tiles of [128q x 512c], alternating 2 PSUM banks
  - fused threshold+count per m-tile on ACT (sign(p+THR) with free-axis
    accum), giving per-query subset clip counts (the collapse
    certificate).  All counts run on ACT because the DVE tensor_scalar
    accum_out path returns the last element instead of the sum on this
    hardware; DVE scalar_tensor_tensor accum (used for the pair dots)
    works correctly.
  - pair-distance path: D_i for its shard of the 3000 links
    (bf16 dot + f32 arccosh chain with a Newton-refined sqrt)
Host: shard/gather + count-gate check + closed-form assembly; exact numpy
fallback if the gate ever fails (makes kernel() total for any input).

Environment notes (this walrus/axon build):
  - walrus rejects >1 sync-wait per instruction ("Too many sync wait
    commands"): _SplitDrainTileContext splits the Tile kernel-tail drain
    into single-wait drains, and _split_multiwait() post-processes any
    remaining multi-wait instruction the same way.
  - there is no NTFF profile hook, so exec_time_ns is unavailable; the
    runner caches the jitted shard_map callable so repeat calls cost only
    host prep + transfer + dispatch (~0.18 s through the axon tunnel).
"""
import numpy as np
import ml_dtypes
from contextlib import ExitStack

import concourse.bass as bass
import concourse.tile as tile
from concourse import mybir

F32 = mybir.dt.float32
BF16 = mybir.dt.bfloat16

N_NODES = 30000
DIM = 128
T_LINKS = 3000
K_NEG = 75
GAMMA = 1.0
EPS = 1e-7
MAX_SQDIST = 50.0

NCORES = 8
NQ = 2 * T_LINKS                  # 6000 queries
QPC = 768                         # queries per core (6144 total, padded)
MT = QPC // 128                   # 6 query tiles per core
NSUB = 512                        # certificate candidate-subset size
SUB_STRIDE = 58                   # 512*58 = 29696 <= 30000
PAIRS = T_LINKS // NCORES         # 375 pairs per core
PT = 3                            # pair tiles (3*128 = 384 >= 375)

THR = np.float32(1.0 + EPS)                        # theta clip point
M_CONST = float(np.arccosh(np.float64(THR)) ** 2)  # collapsed top-k value
GATE = K_NEG + 64                                  # certificate threshold
QELR_W = QPC + NSUB + 2 * PT * 128                 # fused input width (2048)

LAST_EXEC_NS = None


class _SplitDrainTileContext(tile.TileContext):
    """TileContext whose kernel-tail drain is split into single-wait drains.

    This walrus build caps the number of sync-wait commands one instruction
    may carry; the stock tail drain waits on every active proc at once (one
    wait per engine/DMA-queue semaphore) and is rejected with "Too many sync
    wait commands".  A ladder of SP drains with one wait each executes
    sequentially on SP and is equivalent.
    """

    def _drain_and_barrier(self, tick_clock, wait_clock):
        from concourse.vector_clock import ScopedClock, VectorClock
        from concourse.tile_sem_assignment import N_PROCS

        gc = tick_clock.global_clock
        for p in range(N_PROCS):
            t = gc.peek_next(p) - 1
            if t <= 0:
                continue
            part = VectorClock([t if q == p else 0 for q in range(N_PROCS)])
            d = self.nc.sync.drain()
            wait_clock.add_sem_waits(d.ins, ScopedClock({None: part}))
        self.nc.all_engine_barrier()
        popped = self.nc._tile_sem_poison_stack.pop()
        assert popped is self._sem_poison
        self.nc.clear_and_free_semaphores(list(self.sems.allocated().values()))
        self.nc.all_engine_barrier()


def _split_multiwait(nc):
    """Split multi-wait instructions into single-wait same-engine drains.

    The walrus build in this environment rejects instructions carrying more
    than one sync-wait command ("Too many sync wait commands").  Engine
    queues execute in order, so waiting on A at queue slot n and on B at
    slot n+1 is equivalent to waiting on {A, B} at slot n+1: move all but
    the last wait onto fresh Drain instructions inserted just before the
    offender on the same engine.
    """
    import copy as _copy

    fn = nc.m.functions[0]
    template = None
    for b in fn.blocks:
        for j in b.instructions:
            if type(j).__name__ == "InstDrain":
                template = j
                break
        if template is not None:
            break
    if template is None:
        return 0
    n_split = 0
    for b in fn.blocks:
        insts = b.instructions
        idx = 0
        while idx < len(insts):
            i = insts[idx]
            si = i.sync_info
            if si is not None and si.on_wait and len(si.on_wait) > 1:
                waits = list(si.on_wait)
                for k, w in enumerate(waits[:-1]):
                    nd = _copy.deepcopy(template)
                    nd.name = f"{i.name}-wsplit{k}"
                    nd.engine = i.engine
                    nsi = nd.sync_info
                    nsi.on_wait = [w]
                    nsi.on_update = []
                    nd.sync_info = nsi
                    insts.insert(idx, nd)
                    idx += 1
                si.on_wait = [waits[-1]]
                i.sync_info = si
                n_split += 1
            idx += 1
    return n_split


def _build_nc():
    nc = bass.Bass()

    def reg_const(value):
        t = nc.alloc_sbuf_tensor(f"const-f32-{value}", [128, 1], F32)
        nc.gpsimd.memset(t.ap(), value)
        nc.const_aps.aps[(F32, float(value))] = t.ap()

    reg_const(float(THR))
    reg_const(-1.0)
    nc.all_engine_barrier()

    # query shard + candidate subset + pair operands fused in ONE bf16
    # tensor: a single host array / transfer / DMA.  Columns:
    #   [0, QPC)                 Q_shard^T
    #   [QPC, QPC+NSUB)          E_sub^T (Minkowski sign folded)
    #   [QPC+NSUB, +2*PT*128)    pair tiles, col = base + which*PT*128 + t*128 + k
    # (bf16 pair operands add ~2e-5 relative error to the final loss)
    LRBASE = QPC + NSUB
    qelr = nc.dram_tensor("qelr", [128, LRBASE + 2 * PT * 128], BF16,
                          kind="ExternalInput")

    # counts (cols 0..MT-1, ACT) and pair sqdists (cols MT..MT+PT-1, DVE)
    # fused in one output -> one result array through PJRT
    res = nc.dram_tensor("res", [128, MT + PT], F32, kind="ExternalOutput")

    with _SplitDrainTileContext(nc) as tc, ExitStack() as ctx:
        weights = ctx.enter_context(tc.tile_pool(name="weights", bufs=1))
        persist = ctx.enter_context(tc.tile_pool(name="persist", bufs=1))
        dpath = ctx.enter_context(tc.tile_pool(name="dpath", bufs=1))
        scratch = ctx.enter_context(tc.tile_pool(name="scratch", bufs=3))
        psA = ctx.enter_context(tc.tile_pool(name="psA", bufs=2, space="PSUM"))
        psD = ctx.enter_context(tc.tile_pool(name="psD", bufs=2, space="PSUM"))

        qelr_t = weights.tile([128, LRBASE + 2 * PT * 128], BF16)
        nc.sync.dma_start(out=qelr_t, in_=qelr[:, :])
        qT_t = qelr_t[:, :QPC]
        eT_t = qelr_t[:, QPC:QPC + NSUB]

        def lr_tile(which, t):
            c0 = LRBASE + which * PT * 128 + t * 128
            return qelr_t[:, c0:c0 + 128]

        # fused output: per-query clip counts (ACT via sign-sum; the DVE
        # tensor_scalar accum_out path silently returns the last element
        # instead of the sum on this hardware) + pair sqdists (DVE)
        a_out = persist.tile([128, MT + PT], F32, name="res", tag="res")
        a_cnt = a_out[:, :MT]

        # ---------------- D path (one core-shard of pairs) ----------------
        d_t = dpath.tile([128, PT], F32)
        for t in range(PT):
            prod = scratch.tile([128, 128], F32, tag="dprod")
            nc.vector.scalar_tensor_tensor(
                out=prod, in0=lr_tile(0, t), scalar=1.0, in1=lr_tile(1, t),
                op0=mybir.AluOpType.mult, op1=mybir.AluOpType.mult,
                accum_out=d_t[:, t:t + 1],
            )
        th = dpath.tile([128, PT], F32)
        nc.vector.tensor_scalar(out=th, in0=d_t, scalar1=-1.0, scalar2=float(THR),
                                op0=mybir.AluOpType.mult, op1=mybir.AluOpType.max)
        th2 = dpath.tile([128, PT], F32)
        nc.scalar.activation(out=th2, in_=th, func=mybir.ActivationFunctionType.Square)
        s_t = dpath.tile([128, PT], F32)
        nc.scalar.activation(out=s_t, in_=th2,
                             func=mybir.ActivationFunctionType.Sqrt, bias=-1.0)
        # Newton refine sqrt: s <- 0.5*(s + y/s), y = th2-1
        y_t = dpath.tile([128, PT], F32)
        nc.vector.tensor_scalar(out=y_t, in0=th2, scalar1=-1.0, scalar2=None,
                                op0=mybir.AluOpType.add)
        r_t = dpath.tile([128, PT], F32)
        nc.vector.reciprocal(out=r_t, in_=s_t)
        t1 = dpath.tile([128, PT], F32)
        nc.vector.tensor_mul(out=t1, in0=y_t, in1=r_t)
        s2 = dpath.tile([128, PT], F32)
        nc.vector.tensor_add(out=s2, in0=s_t, in1=t1)
        s3 = dpath.tile([128, PT], F32)
        nc.vector.tensor_scalar(out=s3, in0=s2, scalar1=0.5, scalar2=None,
                                op0=mybir.AluOpType.mult)
        u_t = dpath.tile([128, PT], F32)
        nc.vector.tensor_add(out=u_t, in0=th, in1=s3)
        a_t = dpath.tile([128, PT], F32)
        nc.scalar.activation(out=a_t, in_=u_t, func=mybir.ActivationFunctionType.Ln)
        a2 = dpath.tile([128, PT], F32)
        nc.scalar.activation(out=a2, in_=a_t, func=mybir.ActivationFunctionType.Square)
        nc.vector.tensor_scalar(out=a_out[:, MT:], in0=a2,
                                scalar1=float(MAX_SQDIST),
                                scalar2=None, op0=mybir.AluOpType.min)

        # ---------------- certificate: matmul + threshold-count ------------
        for m in range(MT):
            w = qT_t[:, m * 128:(m + 1) * 128]
            if m % 2 == 0:
                p_ps = psA.tile([128, NSUB], F32, name="pa", tag="pa")
            else:
                p_ps = psD.tile([128, NSUB], F32, name="pd", tag="pd")
            nc.tensor.matmul(p_ps, w, eT_t, start=True, stop=True)
            sg = scratch.tile([128, NSUB], BF16, tag="sg")
            nc.scalar.activation(
                out=sg, in_=p_ps,
                func=mybir.ActivationFunctionType.Sign,
                bias=float(THR), scale=1.0,
                accum_out=a_cnt[:, m:m + 1],
            )

        nc.sync.dma_start(out=res[:, :], in_=a_out)
    _split_multiwait(nc)
    return nc


_RUNNER = None


def _make_runner():
    """Build nc once and return a cached callable
    (qelr_global[1024, QPC+NSUB+2*PT*128] bf16)
      -> list of 8 per-core {res} float32 arrays.

    Mirrors concourse.bass_utils.run_bass_kernel_spmd's axon path
    (bass2jax.run_bass_via_pjrt) but hoists the trace/lower/jit out of the
    per-call path so repeat calls skip straight to transfer + execute.
    """
    import jax
    from jax.sharding import Mesh, NamedSharding, PartitionSpec
    from jax.experimental.shard_map import shard_map
    from concourse import bass2jax

    nc = _build_nc()
    bass2jax.install_neuronx_cc_hook()

    partition_name = (nc.partition_id_tensor.name
                      if nc.partition_id_tensor else None)

    in_names, out_names, out_avals, zero_outs = [], [], [], []
    for alloc in nc.m.functions[0].allocations:
        if not isinstance(alloc, mybir.MemoryLocationSet):
            continue
        name = alloc.memorylocations[0].name
        if alloc.kind == "ExternalInput":
            if name != partition_name:
                in_names.append(name)
        elif alloc.kind == "ExternalOutput":
            out_names.append(name)
            shape = tuple(alloc.tensor_shape)
            dtype = mybir.dt.np(alloc.dtype)
            out_avals.append(jax.core.ShapedArray(shape, dtype))
            zero_outs.append(np.zeros((NCORES * shape[0], *shape[1:]), dtype))
    n_params = len(in_names)
    n_outs = len(out_avals)
    all_names = list(in_names) + list(out_names)
    if partition_name is not None:
        all_names.append(partition_name)

    def _body(*args):
        operands = list(args)
        if partition_name is not None:
            operands.append(bass2jax.partition_id_tensor())
        outs = bass2jax._bass_exec_p.bind(
            *operands,
            out_avals=tuple(out_avals),
            in_names=tuple(all_names),
            out_names=tuple(out_names),
            lowering_input_output_aliases=(),
            sim_require_finite=True,
            sim_require_nnan=True,
            nc=nc,
        )
        return tuple(outs)

    devices = jax.devices()[:NCORES]
    assert len(devices) == NCORES
    mesh = Mesh(np.asarray(devices), ("core",))
    spec = PartitionSpec("core")
    in_specs = (spec,) * (n_params + n_outs)
    out_specs = (spec,) * n_outs
    # No donation: both outputs are fully written by the kernel, so the
    # pre-zeroed "output parameter" buffers never need refreshing - keep
    # them resident on device and reuse across calls (saves per-call
    # upload + donation bookkeeping).
    sharded = jax.jit(
        shard_map(_body, mesh=mesh, in_specs=in_specs, out_specs=out_specs,
                  check_rep=False),
        keep_unused=True,
    )
    zdev = [jax.device_put(z, NamedSharding(mesh, spec)) for z in zero_outs]
    jax.block_until_ready(zdev)

    name_to_pos = {n: i for i, n in enumerate(in_names)}

    def run(qelr_global):
        ins = [None] * n_params
        ins[name_to_pos["qelr"]] = qelr_global
        out_arrs = sharded(*ins, *zdev)
        res = []
        for c in range(NCORES):
            res.append({
                name: np.asarray(out_arrs[i]).reshape(
                    NCORES, *out_avals[i].shape)[c]
                for i, name in enumerate(out_names)
            })
        return res

    return run


def _host_fallback(emb, c, links):
    """Exact reference computation on host (safety net).

    sqdist is monotone non-increasing in the Minkowski product p, so the 75
    smallest sqdists are the 75 largest p: select them with an O(N) f32
    partition, then evaluate the arccosh chain in f64 on just those.
    Bit-identical to the full f64 sort on the reference inputs, ~24x faster
    (~2 s vs ~50 s).
    """
    cs = np.float64(c[0])
    L = emb[links[:, 0]].astype(np.float64)
    R = emb[links[:, 1]].astype(np.float64)
    K = 1.0 / cs

    def sqd(prod):
        theta = np.maximum(-prod / K, 1.0 + EPS)
        return np.minimum(K * np.arccosh(theta) ** 2, MAX_SQDIST)

    d = -L[:, 0] * R[:, 0] + (L[:, 1:] * R[:, 1:]).sum(1)
    D = sqd(d) + GAMMA
    embp32 = emb.copy()
    embp32[:, 0] = -embp32[:, 0]
    total = 0.0
    for Q32 in (emb[links[:, 0]], emb[links[:, 1]]):
        P32 = Q32 @ embp32.T                                   # (3000, 30000)
        topp = -np.partition(-P32, K_NEG - 1, axis=1)[:, :K_NEG]
        S = sqd(topp.astype(np.float64))
        total += np.maximum(D[:, None] - S, 0.0).sum()
    return np.float32(total / (2.0 * K_NEG * T_LINKS))


def kernel(embeddings, c, train_links):
    global _RUNNER, LAST_EXEC_NS
    emb = np.asarray(embeddings, dtype=np.float32)
    cc = np.asarray(c, dtype=np.float32)
    links = np.asarray(train_links)

    if abs(float(cc[0]) - 1.0) > 1e-12:
        return _host_fallback(emb, cc, links)

    # ---- host-side sharding / layout prep
    L = emb[links[:, 0]]                       # (3000, 128)
    R = emb[links[:, 1]]

    # certificate operands: per-core query shard (bf16) + shared subset
    e_sub = emb[np.arange(NSUB) * SUB_STRIDE].copy()   # (512, 128)
    e_sub[:, 0] = -e_sub[:, 0]                         # fold Minkowski sign
    e_sub_T = np.ascontiguousarray(e_sub.T).astype(ml_dtypes.bfloat16)

    # queries [L; R] padded to 8*QPC, cast to bf16 while contiguous so the
    # transposed scatter below moves 2-byte elements with no per-element cast
    Qp = np.zeros((NCORES * QPC, DIM), np.float32)
    Qp[:T_LINKS] = L
    Qp[T_LINKS:NQ] = R
    Qb = Qp.astype(ml_dtypes.bfloat16)
    # (8, 128, QPC): core c gets queries [c*QPC, (c+1)*QPC) transposed
    QT = Qb.reshape(NCORES, QPC, DIM).transpose(0, 2, 1)

    # D-path operands: [c, p, which, t, k] = (Lp|R)[c*375+t*128+p, k]
    # (scatter casts f32 -> bf16 once; the big transpose below is 2B -> 2B)
    Lp = L.copy()
    Lp[:, 0] = -Lp[:, 0]
    lr_pad = np.zeros((2, NCORES, PT, 128, DIM), ml_dtypes.bfloat16)
    lr_pad[0].reshape(-1, DIM)[_pair_scatter_idx()] = Lp
    lr_pad[1].reshape(-1, DIM)[_pair_scatter_idx()] = R

    # single fused input array
    qelr_global = np.empty((NCORES * 128, QELR_W), ml_dtypes.bfloat16)
    q3 = qelr_global.reshape(NCORES, 128, QELR_W)
    q3[:, :, :QPC] = QT
    q3[:, :, QPC:QPC + NSUB] = e_sub_T[None]
    q3[:, :, QPC + NSUB:] = lr_pad.transpose(1, 3, 0, 2, 4).reshape(
        NCORES, 128, 2 * PT * 128)

    try:
        if _RUNNER is None:
            _RUNNER = _make_runner()
        results = _RUNNER(qelr_global)
    except Exception:
        return _host_fallback(emb, cc, links)
    LAST_EXEC_NS = None

    # ---- unshard / assemble
    sq_sum = 0.0
    cnt_min = np.inf
    for core in range(NCORES):
        r = results[core]["res"]
        # sign-sum to count: cnt = (NSUB + sum_sign)/2
        cnt = (float(NSUB) + r[:, :MT].astype(np.float64)) / 2.0  # (128, MT)
        qbase = core * QPC
        nvalid = min(max(NQ - qbase, 0), QPC)
        if nvalid > 0:
            valid = cnt.T.reshape(-1)[:nvalid]
            cnt_min = min(cnt_min, valid.min())
        s = r[:, MT:].astype(np.float64).T.reshape(-1)[:PAIRS]
        sq_sum += s.sum()

    if cnt_min < GATE:
        # top-k collapse not certified for some query -> exact fallback
        return _host_fallback(emb, cc, links)

    loss = sq_sum / T_LINKS + GAMMA - M_CONST  # mean(D) - m
    return np.float32(loss)


_PAIR_IDX = None


def _pair_scatter_idx():
    """Flat indices into (NCORES*PT*128) for pair i -> core i//375,
    tile (i%375)//128, partition (i%375)%128."""
    global _PAIR_IDX
    if _PAIR_IDX is None:
        i = np.arange(T_LINKS)
        core, rem = i // PAIRS, i % PAIRS
        _PAIR_IDX = core * (PT * 128) + rem
    return _PAIR_IDX
